# revision 5
# baseline (speedup 1.0000x reference)
"""Trainium2 Bass kernel for nn_ConstrainNet (block-banded dynamics residual).

Reference computation (n_state=64, n_input=32, n_all=96, T=128):
    V = net_input.reshape(T, 96)
    out block 0      = V[0, :64] - x0
    out block t+1    = [A B] @ V[t] - V[t+1, :64]        (t = 0..T-2)
    output = concat of the 128 blocks -> (8192,) f32

Sharding: time axis split across 8 NeuronCores; core k computes output
blocks t in [16k, 16k+16). Inputs arrive FULL on host, so the one-step
"halo" is just an overlapping host-side slice — no collectives.

The whole per-core computation is folded into ONE matmul with an
augmented contraction dimension K = 96 + 1 + 16 = 113:
    out[j, s] = sum_a lhsT[a, j] * rhs[a, s]
      rows  0..95 : lhsT = Vm^T, rhs = [A B]^T          -> AB @ Vm[j]
      row     96  : identity-block fixup, used by core 0 only:
                    lhsT[96, 0] = 1, rhs[96, :] = V[0, :64]
      rows 97..112: lhsT[97+j', j] = -delta(j', j), rhs[97+j] = S[j]
                    -> subtracts S[j] (= V[t+1, :64], or x0 for block 0)
All augmentation entries are constants or pure host-side slices — no
host arithmetic. Device program is a 1-dep chain:
    DMA(w) -> matmul -> copy psum->sbuf -> DMA(out)
(each instruction carries at most one sync wait; walrus on this
toolchain rejects instructions with more).
"""

import numpy as np

N_STATE = 64
N_INPUT = 32
N_ALL = N_STATE + N_INPUT  # 96
T_FULL = 128
N_CORES = 8
TB = T_FULL // N_CORES  # 16 output blocks per core
K = N_ALL + 1 + TB  # 113 contraction rows
W_COLS = N_STATE + TB  # 80: [rhs | lhsT] packed along free dim

_PROGRAM_CACHE = {}


def _build_program():
    import concourse.bass as bass
    import concourse.mybir as mybir

    f32 = mybir.dt.float32
    nc = bass.Bass("TRN2", debug=False)

    w = nc.dram_tensor("w", [K, W_COLS], f32, kind="ExternalInput")
    out_d = nc.dram_tensor("out", [TB, N_STATE], f32, kind="ExternalOutput")

    # Raw Bass (no TileContext): this walrus build rejects instructions
    # carrying more than one sync wait, and Tile's end-of-context drain
    # aggregates one wait per live semaphore. The manual chain below has
    # at most one wait per instruction.
    with (
        nc.sbuf_tensor([K, W_COLS], f32) as w_t,
        nc.psum_tensor([TB, N_STATE], f32) as acc,
        nc.sbuf_tensor([TB, N_STATE], f32) as o_t,
        nc.semaphore("dma_in") as dma_in,
        nc.semaphore("mm") as mm,
        nc.semaphore("cp") as cp,
        nc.semaphore("dma_out") as dma_out,
        nc.Block() as block,
    ):

        @block.sync
        def _(sp):
            sp.dma_start(out=w_t[:], in_=w[:]).then_inc(dma_in, 16)
            sp.wait_ge(cp, 1)
            sp.dma_start(out=out_d[:], in_=o_t[:]).then_inc(dma_out, 16)
            sp.wait_ge(dma_out, 16)

        @block.tensor
        def _(pe):
            pe.wait_ge(dma_in, 16)
            pe.matmul(
                acc[:],
                w_t[:, N_STATE:W_COLS],
                w_t[:, 0:N_STATE],
                start=True,
                stop=True,
            ).then_inc(mm, 1)

        @block.vector
        def _(dve):
            dve.wait_ge(mm, 1)
            dve.tensor_copy(o_t[:], acc[:]).then_inc(cp, 1)

    return nc


def _get_program():
    if "nc" not in _PROGRAM_CACHE:
        _PROGRAM_CACHE["nc"] = _build_program()
    return _PROGRAM_CACHE["nc"]


def _make_in_maps(A, B, x0, net_input):
    A = np.ascontiguousarray(A, dtype=np.float32)
    B = np.ascontiguousarray(B, dtype=np.float32)
    x0 = np.ascontiguousarray(x0, dtype=np.float32)
    V = np.ascontiguousarray(net_input, dtype=np.float32).reshape(T_FULL, N_ALL)

    ab_t = np.concatenate([A, B], axis=1).T  # (96, 64)

    in_maps = []
    for k in range(N_CORES):
        w = np.zeros((K, W_COLS), dtype=np.float32)
        rhs = w[:, :N_STATE]
        lhsT = w[:, N_STATE:]
        rhs[:N_ALL] = ab_t
        # rows 97..112: -I in lhsT, S rows in rhs
        lhsT[N_ALL + 1 :] = -np.eye(TB, dtype=np.float32)
        t0 = k * TB
        if k == 0:
            rhs[N_ALL] = V[0, :N_STATE]  # identity-block fixup
            lhsT[N_ALL, 0] = 1.0
            lhsT[:N_ALL, 1:] = V[0 : TB - 1].T
            rhs[N_ALL + 1] = x0
            rhs[N_ALL + 2 :] = V[1:TB, :N_STATE]
        else:
            lhsT[:N_ALL] = V[t0 - 1 : t0 + TB - 1].T
            rhs[N_ALL + 1 :] = V[t0 : t0 + TB, :N_STATE]
        in_maps.append({"w": w})
    return in_maps


def kernel(A, B, x0, net_input, T):
    assert int(T) == T_FULL, f"kernel hardcoded for T={T_FULL}, got {T}"
    from concourse.bass_utils import run_bass_kernel_spmd

    nc = _get_program()
    in_maps = _make_in_maps(A, B, x0, net_input)
    res = run_bass_kernel_spmd(nc, in_maps, core_ids=list(range(N_CORES)))
    out = np.concatenate([np.asarray(r["out"]).reshape(-1) for r in res.results])
    return out.astype(np.float32)
